# revision 9
# baseline (speedup 1.0000x reference)
"""MoE block (E=8 experts, top-2 routing, SwiGLU experts) on 8 Trainium2 cores.

Strategy (expert-parallel):
  - Routing (gate logits, top-2, softmax combine weights) is computed on host
    in float64: the gate matmul is only N*D*E = 67M MACs (0.04% of total
    FLOPs), while the expert FFN is 196G MACs.  Rank-2/3 logit margins are
    >5e-5 for this problem size, orders of magnitude above fp32 rounding
    noise, so fp64 host routing reproduces the fp32 reference routing
    exactly.
  - Core j receives the tokens routed to expert j (gathered, transposed to
    feature-major, bf16) plus expert j's weights pre-transposed/packed so
    every DMA is contiguous and every matmul operand is already in lhsT/rhs
    layout.  All matmuls run in bf16 with fp32 PSUM accumulation.
  - Device computes e_j = silu(h @ wg.T + bg) * (h @ wv.T + bv) with
    h = x @ w1.T + b1, feature-major (tokens on the free dim).  The combine
    (scale by softmax weight, scatter-add over the two experts per token)
    happens on host in fp32.
"""

import math
import os
from contextlib import ExitStack

import ml_dtypes
import numpy as np

import concourse.bass as bass
import concourse.mybir as mybir
import concourse.tile as tile
from concourse import bacc
from concourse.bass_utils import run_bass_kernel_spmd

D = 1024
E = 8
K = 2
R = 16
ALPHA = 32.0
SCALING = ALPHA / R
H = 4096
P = 128
NSUB = 512  # matmul moving-operand (token) tile
DK = D // P   # 8 contraction tiles of 128 over D
HK = H // P   # 32 contraction tiles of 128 over H

BF16 = mybir.dt.bfloat16
FP32 = mybir.dt.float32
AF = mybir.ActivationFunctionType
np_bf16 = ml_dtypes.bfloat16

# Minimum token capacity per expert core (multiple of 128).  Expected load is
# N*K/E = 2048 +- ~45; 2176 covers it.  If an unusual input routes more
# tokens to one expert, we transparently rebuild for a larger C.
C_MIN = 2176

_program_cache: dict[int, "bass.Bass"] = {}

# Populated by the most recent kernel() call when MOE_TRACE=1: BassKernelResults
last_results = None
last_exec_time_ns = None


def _chunks(C):
    """Split C (multiple of 128) into token chunks of at most NSUB."""
    out = []
    c0 = 0
    while c0 < C:
        cw = min(NSUB, C - c0)
        out.append((c0, cw))
        c0 += cw
    return out


def _build_program(C):
    """One expert's FFN over C (padded) tokens, feature-major layouts.

    DRAM inputs (per core):
      xT  [DK, 128, C]   bf16  xT[k, d, c]   = x_tokens[c, k*128+d]
      w1p [HK, 128, D]   bf16  w1p[ht,d,k*128+h] = w1[ht*128+h, k*128+d]
      wgp [128, HK*D]    bf16  wgp[h, k*D+d]     = wg[d, k*128+h]
      wvp [DK, 128, H]   bf16  wvp[dt,h,k*128+d] = wv[dt*128+d, k*128+h]
      b1p [128, HK] fp32, bgp/bvp [128, DK] fp32 (per-partition bias columns)
    DRAM output:
      outT [DK, 128, C] fp32   outT[dt, d, c] = e[c, dt*128+d]
    """
    # Bacc (not raw Bass): its compile() pipeline splits multi-wait sync_info
    # into event semaphores — TRN2 instructions support at most one wait, and
    # walrus codegen rejects Tile's multi-wait output otherwise.
    nc = bacc.Bacc("TRN2", target_bir_lowering=False, debug=False)
    xT_d = nc.dram_tensor("xT", [DK, P, C], BF16, kind="ExternalInput")
    w1_d = nc.dram_tensor("w1p", [HK, P, D], BF16, kind="ExternalInput")
    wg_d = nc.dram_tensor("wgp", [P, HK * D], BF16, kind="ExternalInput")
    wv_d = nc.dram_tensor("wvp", [DK, P, H], BF16, kind="ExternalInput")
    b1_d = nc.dram_tensor("b1p", [P, HK], FP32, kind="ExternalInput")
    bg_d = nc.dram_tensor("bgp", [P, DK], FP32, kind="ExternalInput")
    bv_d = nc.dram_tensor("bvp", [P, DK], FP32, kind="ExternalInput")
    out_d = nc.dram_tensor("outT", [DK, P, C], FP32, kind="ExternalOutput")

    with tile.TileContext(nc) as tc, ExitStack() as ctx:
        const = ctx.enter_context(tc.tile_pool(name="const", bufs=1))
        wg_sb = const.tile([P, HK * D], BF16)  # 64KB/partition, resident
        nc.sync.dma_start(out=wg_sb[:, :], in_=wg_d[:, :])
        b1_sb = const.tile([P, HK], FP32)
        nc.sync.dma_start(out=b1_sb[:, :], in_=b1_d[:, :])
        bg_sb = const.tile([P, DK], FP32)
        nc.sync.dma_start(out=bg_sb[:, :], in_=bg_d[:, :])
        bv_sb = const.tile([P, DK], FP32)
        nc.sync.dma_start(out=bv_sb[:, :], in_=bv_d[:, :])

        xpool = ctx.enter_context(tc.tile_pool(name="x", bufs=2))
        w1pool = ctx.enter_context(tc.tile_pool(name="w1", bufs=4))
        wvpool = ctx.enter_context(tc.tile_pool(name="wv", bufs=3))
        hpool = ctx.enter_context(tc.tile_pool(name="h", bufs=1))
        gvpool = ctx.enter_context(tc.tile_pool(name="gv", bufs=3))
        opool = ctx.enter_context(tc.tile_pool(name="o", bufs=4))
        ps1 = ctx.enter_context(tc.tile_pool(name="ps1", bufs=2, space="PSUM"))
        ps2 = ctx.enter_context(tc.tile_pool(name="ps2", bufs=2, space="PSUM"))

        for c0, cw in _chunks(C):
            x_sb = xpool.tile([P, DK * cw], BF16, tag="x")
            for k in range(DK):
                nc.sync.dma_start(
                    out=x_sb[:, k * cw : (k + 1) * cw], in_=xT_d[k, :, c0 : c0 + cw]
                )

            # Stage 1: hT[h, c] = sum_k w1T[k-block](d,h).T @ xT[k-block](d,c)
            h_sb = hpool.tile([P, HK * cw], BF16, tag="h")
            for ht in range(HK):
                w1_sb = w1pool.tile([P, D], BF16, tag="w1")
                nc.sync.dma_start(out=w1_sb[:, :], in_=w1_d[ht, :, :])
                pt = ps1.tile([P, cw], FP32, tag="pt")
                for k in range(DK):
                    nc.tensor.matmul(
                        pt[:, :],
                        w1_sb[:, k * P : (k + 1) * P],
                        x_sb[:, k * cw : (k + 1) * cw],
                        start=(k == 0),
                        stop=(k == DK - 1),
                    )
                nc.scalar.activation(
                    h_sb[:, ht * cw : (ht + 1) * cw],
                    pt[:, :],
                    AF.Identity,
                    bias=b1_sb[:, ht : ht + 1],
                )

            # Stage 2: per D-tile, g = silu(wg@h + bg), v = wv@h + bv, e = g*v
            for dt in range(DK):
                wv_sb = wvpool.tile([P, H], BF16, tag="wv")
                nc.sync.dma_start(out=wv_sb[:, :], in_=wv_d[dt, :, :])

                pg = ps2.tile([P, cw], FP32, tag="pg")
                for k in range(HK):
                    nc.tensor.matmul(
                        pg[:, :],
                        wg_sb[:, k * D + dt * P : k * D + (dt + 1) * P],
                        h_sb[:, k * cw : (k + 1) * cw],
                        start=(k == 0),
                        stop=(k == HK - 1),
                    )
                # silu(u) = u * sigmoid(u), u = pg + bg  (CoreSim lacks Silu)
                s_sb = gvpool.tile([P, cw], FP32, tag="s")
                nc.scalar.activation(
                    s_sb[:, :], pg[:, :], AF.Sigmoid, bias=bg_sb[:, dt : dt + 1]
                )
                g_sb = gvpool.tile([P, cw], FP32, tag="g")
                nc.vector.scalar_tensor_tensor(
                    g_sb[:, :],
                    pg[:, :],
                    bg_sb[:, dt : dt + 1],
                    s_sb[:, :],
                    mybir.AluOpType.add,
                    mybir.AluOpType.mult,
                )

                pv = ps2.tile([P, cw], FP32, tag="pv")
                for k in range(HK):
                    nc.tensor.matmul(
                        pv[:, :],
                        wv_sb[:, k * P : (k + 1) * P],
                        h_sb[:, k * cw : (k + 1) * cw],
                        start=(k == 0),
                        stop=(k == HK - 1),
                    )
                v_sb = gvpool.tile([P, cw], FP32, tag="v")
                nc.scalar.activation(
                    v_sb[:, :], pv[:, :], AF.Identity, bias=bv_sb[:, dt : dt + 1]
                )

                e_sb = opool.tile([P, cw], FP32, tag="e")
                nc.vector.tensor_mul(e_sb[:, :], g_sb[:, :], v_sb[:, :])
                nc.sync.dma_start(out=out_d[dt, :, c0 : c0 + cw], in_=e_sb[:, :])

    return nc


def _get_program(C):
    if C not in _program_cache:
        nc = _build_program(C)
        nc.finalize()  # runs Bacc.compile(): wait splitting, reg alloc, DCE
        _program_cache[C] = nc
    return _program_cache[C]


def _route(x, task_id_tensor, task_emb, base_gate_w, lora_A, lora_B):
    """Host routing.  Returns (x_flat fp32, per-expert ids, per-expert cw)."""
    x = np.asarray(x, dtype=np.float32)
    tid = np.asarray(task_id_tensor).astype(np.int64).reshape(-1)
    task_emb = np.asarray(task_emb, dtype=np.float32)
    x_flat = x.reshape(-1, D) + task_emb[tid]

    w_eff = (
        np.asarray(base_gate_w, dtype=np.float64)
        + SCALING
        * (np.asarray(lora_A, dtype=np.float64) @ np.asarray(lora_B, dtype=np.float64)).T
    )
    logits = x_flat.astype(np.float64) @ w_eff.T  # [N, E]

    n = logits.shape[0]
    rows = np.arange(n)
    i1 = logits.argmax(axis=1)
    v1 = logits[rows, i1]
    masked = logits.copy()
    masked[rows, i1] = -np.inf
    i2 = masked.argmax(axis=1)
    v2 = masked[rows, i2]
    # softmax over the two selected logits (v1 >= v2)
    t = np.exp(v2 - v1)
    w1 = (1.0 / (1.0 + t)).astype(np.float32)
    w2 = (t / (1.0 + t)).astype(np.float32)

    ids, cws = [], []
    for j in range(E):
        m1 = i1 == j
        m2 = i2 == j
        idx = np.concatenate([rows[m1], rows[m2]])
        cw = np.concatenate([w1[m1], w2[m2]])
        ids.append(idx)
        cws.append(cw)
    return x_flat, ids, cws


def _pack_core_inputs(x_flat, ids_j, w1j, wgj, wvj, b1j, bgj, bvj, C):
    """Build the per-core in_map for one expert."""
    cnt = len(ids_j)
    xe = np.zeros((C, D), dtype=np_bf16)
    xe[:cnt] = x_flat[ids_j].astype(np_bf16)
    xT = np.ascontiguousarray(xe.T).reshape(DK, P, C)

    w1p = np.ascontiguousarray(
        w1j.reshape(HK, P, DK, P).transpose(0, 3, 2, 1).astype(np_bf16)
    ).reshape(HK, P, D)
    wgp = np.ascontiguousarray(
        wgj.reshape(D, HK, P).transpose(2, 1, 0).astype(np_bf16)
    ).reshape(P, HK * D)
    wvp = np.ascontiguousarray(
        wvj.reshape(DK, P, HK, P).transpose(0, 3, 2, 1).astype(np_bf16)
    ).reshape(DK, P, H)
    b1p = np.ascontiguousarray(b1j.reshape(HK, P).T.astype(np.float32))
    bgp = np.ascontiguousarray(bgj.reshape(DK, P).T.astype(np.float32))
    bvp = np.ascontiguousarray(bvj.reshape(DK, P).T.astype(np.float32))
    return dict(xT=xT, w1p=w1p, wgp=wgp, wvp=wvp, b1p=b1p, bgp=bgp, bvp=bvp)


def kernel(
    x,
    task_id_tensor,
    task_emb,
    base_gate_w,
    lora_A,
    lora_B,
    w1,
    b1,
    wg,
    bg,
    wv,
    bv,
):
    global last_results, last_exec_time_ns
    x = np.asarray(x)
    bsz, seqlen, dim = x.shape
    assert dim == D

    x_flat, ids, cws = _route(x, task_id_tensor, task_emb, base_gate_w, lora_A, lora_B)

    max_cnt = max(len(i) for i in ids)
    C = max(C_MIN, ((max_cnt + P - 1) // P) * P)
    nc = _get_program(C)

    w1 = np.asarray(w1, dtype=np.float32)
    b1 = np.asarray(b1, dtype=np.float32)
    wg = np.asarray(wg, dtype=np.float32)
    bg = np.asarray(bg, dtype=np.float32)
    wv = np.asarray(wv, dtype=np.float32)
    bv = np.asarray(bv, dtype=np.float32)

    in_maps = [
        _pack_core_inputs(x_flat, ids[j], w1[j], wg[j], wv[j], b1[j], bg[j], bv[j], C)
        for j in range(E)
    ]

    trace = os.environ.get("MOE_TRACE", "0") == "1"
    try:
        res = run_bass_kernel_spmd(nc, in_maps, list(range(E)), trace=trace)
    except (ImportError, ModuleNotFoundError):
        # axon NTFF profiling hook unavailable in this container
        res = run_bass_kernel_spmd(nc, in_maps, list(range(E)), trace=False)
    last_results = res
    last_exec_time_ns = getattr(res, "exec_time_ns", None)

    out_flat = np.zeros((bsz * seqlen, D), dtype=np.float32)
    for j in range(E):
        cnt = len(ids[j])
        if cnt == 0:
            continue
        e = np.asarray(res.results[j]["outT"]).reshape(D, C)[:, :cnt].T
        out_flat[ids[j]] += cws[j][:, None] * e
    return out_flat.reshape(bsz, seqlen, dim)


# revision 11
# speedup vs baseline: 3.3093x; 3.3093x over previous
"""MoE block (E=8 experts, top-2 routing, SwiGLU experts) on 8 Trainium2 cores.

Strategy (expert-parallel):
  - Routing (gate logits, top-2, softmax combine weights) is computed on host
    in float64: the gate matmul is only N*D*E = 67M MACs (0.04% of total
    FLOPs), while the expert FFN is 196G MACs.  Rank-2/3 logit margins are
    >5e-5 for this problem size, orders of magnitude above fp32 rounding
    noise, so fp64 host routing reproduces the fp32 reference routing
    exactly.
  - Core j receives the tokens routed to expert j (gathered, transposed to
    feature-major, bf16) plus expert j's weights pre-transposed/packed so
    every DMA is contiguous and every matmul operand is already in lhsT/rhs
    layout.  All matmuls run in bf16 with fp32 PSUM accumulation.
  - Device computes e_j = silu(h @ wg.T + bg) * (h @ wv.T + bv) with
    h = x @ w1.T + b1, feature-major (tokens on the free dim).  The combine
    (scale by softmax weight, scatter-add over the two experts per token)
    happens on host in fp32.
"""

import math
import os
from contextlib import ExitStack

import ml_dtypes
import numpy as np

import concourse.bass as bass
import concourse.mybir as mybir
import concourse.tile as tile
from concourse import bacc
from concourse.bass_utils import run_bass_kernel_spmd

D = 1024
E = 8
K = 2
R = 16
ALPHA = 32.0
SCALING = ALPHA / R
H = 4096
P = 128
NSUB = 448  # matmul moving-operand (token) tile; 2176 = 4*448 + 384 keeps
# every chunk >= 384 wide (uniform PE efficiency, LDWEIGHTS fully hidden)
DK = D // P   # 8 contraction tiles of 128 over D
HK = H // P   # 32 contraction tiles of 128 over H

BF16 = mybir.dt.bfloat16
FP32 = mybir.dt.float32
AF = mybir.ActivationFunctionType
np_bf16 = ml_dtypes.bfloat16

# Minimum token capacity per expert core (multiple of 128).  Expected load is
# N*K/E = 2048 +- ~45; 2176 covers it.  If an unusual input routes more
# tokens to one expert, we transparently rebuild for a larger C.
C_MIN = 2176

_program_cache: dict[int, "bass.Bass"] = {}

# Populated by the most recent kernel() call when MOE_TRACE=1: BassKernelResults
last_results = None
last_exec_time_ns = None


def _chunks(C):
    """Split C (multiple of 128) into token chunks of at most NSUB."""
    out = []
    c0 = 0
    while c0 < C:
        cw = min(NSUB, C - c0)
        out.append((c0, cw))
        c0 += cw
    return out


def _build_program(C):
    """One expert's FFN over C (padded) tokens, feature-major layouts.

    DRAM inputs (per core):
      xT  [DK, 128, C]   bf16  xT[k, d, c]   = x_tokens[c, k*128+d]
      w1p [HK, 128, D]   bf16  w1p[ht,d,k*128+h] = w1[ht*128+h, k*128+d]
      wgp [128, HK*D]    bf16  wgp[h, k*D+d]     = wg[d, k*128+h]
      wvp [DK, 128, H]   bf16  wvp[dt,h,k*128+d] = wv[dt*128+d, k*128+h]
      b1p [128, HK] fp32, bgp/bvp [128, DK] fp32 (per-partition bias columns)
    DRAM output:
      outT [DK, 128, C] fp32   outT[dt, d, c] = e[c, dt*128+d]
    """
    # Bacc (not raw Bass): its compile() pipeline splits multi-wait sync_info
    # into event semaphores — TRN2 instructions support at most one wait, and
    # walrus codegen rejects Tile's multi-wait output otherwise.
    nc = bacc.Bacc("TRN2", target_bir_lowering=False, debug=False)
    xT_d = nc.dram_tensor("xT", [DK, P, C], BF16, kind="ExternalInput")
    w1_d = nc.dram_tensor("w1p", [HK, P, D], BF16, kind="ExternalInput")
    wg_d = nc.dram_tensor("wgp", [P, HK * D], BF16, kind="ExternalInput")
    wv_d = nc.dram_tensor("wvp", [DK, P, H], BF16, kind="ExternalInput")
    b1_d = nc.dram_tensor("b1p", [P, HK], FP32, kind="ExternalInput")
    bg_d = nc.dram_tensor("bgp", [P, DK], FP32, kind="ExternalInput")
    bv_d = nc.dram_tensor("bvp", [P, DK], FP32, kind="ExternalInput")
    out_d = nc.dram_tensor("outT", [DK, P, C], FP32, kind="ExternalOutput")

    with tile.TileContext(nc) as tc, ExitStack() as ctx:
        const = ctx.enter_context(tc.tile_pool(name="const", bufs=1))
        wg_sb = const.tile([P, HK * D], BF16)  # 64KB/partition, resident
        nc.sync.dma_start(out=wg_sb[:, :], in_=wg_d[:, :])
        b1_sb = const.tile([P, HK], FP32)
        nc.sync.dma_start(out=b1_sb[:, :], in_=b1_d[:, :])
        bg_sb = const.tile([P, DK], FP32)
        nc.sync.dma_start(out=bg_sb[:, :], in_=bg_d[:, :])
        bv_sb = const.tile([P, DK], FP32)
        nc.sync.dma_start(out=bv_sb[:, :], in_=bv_d[:, :])

        xpool = ctx.enter_context(tc.tile_pool(name="x", bufs=2))
        w1pool = ctx.enter_context(tc.tile_pool(name="w1", bufs=4))
        wvpool = ctx.enter_context(tc.tile_pool(name="wv", bufs=2))
        hpool = ctx.enter_context(tc.tile_pool(name="h", bufs=2))
        gvpool = ctx.enter_context(tc.tile_pool(name="gv", bufs=3))
        opool = ctx.enter_context(tc.tile_pool(name="o", bufs=4))
        ps1 = ctx.enter_context(tc.tile_pool(name="ps1", bufs=3, space="PSUM"))
        ps2 = ctx.enter_context(tc.tile_pool(name="ps2", bufs=2, space="PSUM"))

        for c0, cw in _chunks(C):
            x_sb = xpool.tile([P, DK * cw], BF16, tag="x")
            for k in range(DK):
                nc.sync.dma_start(
                    out=x_sb[:, k * cw : (k + 1) * cw], in_=xT_d[k, :, c0 : c0 + cw]
                )

            # Stage 1: hT[h, c] = sum_k w1T[k-block](d,h).T @ xT[k-block](d,c)
            h_sb = hpool.tile([P, HK * cw], BF16, tag="h")
            for ht in range(HK):
                w1_sb = w1pool.tile([P, D], BF16, tag="w1")
                nc.sync.dma_start(out=w1_sb[:, :], in_=w1_d[ht, :, :])
                pt = ps1.tile([P, cw], FP32, tag="pt")
                for k in range(DK):
                    nc.tensor.matmul(
                        pt[:, :],
                        w1_sb[:, k * P : (k + 1) * P],
                        x_sb[:, k * cw : (k + 1) * cw],
                        start=(k == 0),
                        stop=(k == DK - 1),
                    )
                nc.scalar.activation(
                    h_sb[:, ht * cw : (ht + 1) * cw],
                    pt[:, :],
                    AF.Identity,
                    bias=b1_sb[:, ht : ht + 1],
                )

            # Stage 2: per D-tile, g = silu(wg@h + bg), v = wv@h + bv, e = g*v
            for dt in range(DK):
                wv_sb = wvpool.tile([P, H], BF16, tag="wv")
                nc.sync.dma_start(out=wv_sb[:, :], in_=wv_d[dt, :, :])

                pg = ps2.tile([P, cw], FP32, tag="pg")
                for k in range(HK):
                    nc.tensor.matmul(
                        pg[:, :],
                        wg_sb[:, k * D + dt * P : k * D + (dt + 1) * P],
                        h_sb[:, k * cw : (k + 1) * cw],
                        start=(k == 0),
                        stop=(k == HK - 1),
                    )
                # silu(u) = u * sigmoid(u), u = pg + bg  (CoreSim lacks Silu)
                s_sb = gvpool.tile([P, cw], FP32, tag="s")
                nc.scalar.activation(
                    s_sb[:, :], pg[:, :], AF.Sigmoid, bias=bg_sb[:, dt : dt + 1]
                )
                g_sb = gvpool.tile([P, cw], FP32, tag="g")
                nc.vector.scalar_tensor_tensor(
                    g_sb[:, :],
                    pg[:, :],
                    bg_sb[:, dt : dt + 1],
                    s_sb[:, :],
                    mybir.AluOpType.add,
                    mybir.AluOpType.mult,
                )

                pv = ps2.tile([P, cw], FP32, tag="pv")
                for k in range(HK):
                    nc.tensor.matmul(
                        pv[:, :],
                        wv_sb[:, k * P : (k + 1) * P],
                        h_sb[:, k * cw : (k + 1) * cw],
                        start=(k == 0),
                        stop=(k == HK - 1),
                    )
                v_sb = gvpool.tile([P, cw], FP32, tag="v")
                nc.scalar.activation(
                    v_sb[:, :], pv[:, :], AF.Identity, bias=bv_sb[:, dt : dt + 1]
                )

                e_sb = opool.tile([P, cw], FP32, tag="e")
                nc.vector.tensor_mul(e_sb[:, :], g_sb[:, :], v_sb[:, :])
                nc.sync.dma_start(out=out_d[dt, :, c0 : c0 + cw], in_=e_sb[:, :])

    return nc


def _get_program(C):
    if C not in _program_cache:
        nc = _build_program(C)
        nc.finalize()  # runs Bacc.compile(): wait splitting, reg alloc, DCE
        _program_cache[C] = nc
    return _program_cache[C]


def _route(x, task_id_tensor, task_emb, base_gate_w, lora_A, lora_B):
    """Host routing.  Returns (x_flat fp32, per-expert ids, per-expert cw)."""
    x = np.asarray(x, dtype=np.float32)
    tid = np.asarray(task_id_tensor).astype(np.int64).reshape(-1)
    task_emb = np.asarray(task_emb, dtype=np.float32)
    x_flat = x.reshape(-1, D) + task_emb[tid]

    w_eff = (
        np.asarray(base_gate_w, dtype=np.float64)
        + SCALING
        * (np.asarray(lora_A, dtype=np.float64) @ np.asarray(lora_B, dtype=np.float64)).T
    )
    logits = x_flat.astype(np.float64) @ w_eff.T  # [N, E]

    n = logits.shape[0]
    rows = np.arange(n)
    i1 = logits.argmax(axis=1)
    v1 = logits[rows, i1]
    masked = logits.copy()
    masked[rows, i1] = -np.inf
    i2 = masked.argmax(axis=1)
    v2 = masked[rows, i2]
    # softmax over the two selected logits (v1 >= v2)
    t = np.exp(v2 - v1)
    w1 = (1.0 / (1.0 + t)).astype(np.float32)
    w2 = (t / (1.0 + t)).astype(np.float32)

    ids, cws = [], []
    for j in range(E):
        m1 = i1 == j
        m2 = i2 == j
        idx = np.concatenate([rows[m1], rows[m2]])
        cw = np.concatenate([w1[m1], w2[m2]])
        ids.append(idx)
        cws.append(cw)
    return x_flat, ids, cws


def _pack_core_inputs(x_flat, ids_j, w1j, wgj, wvj, b1j, bgj, bvj, C):
    """Build the per-core in_map for one expert."""
    cnt = len(ids_j)
    xe = np.zeros((C, D), dtype=np_bf16)
    xe[:cnt] = x_flat[ids_j].astype(np_bf16)
    xT = np.ascontiguousarray(xe.T).reshape(DK, P, C)

    w1p = np.ascontiguousarray(
        w1j.reshape(HK, P, DK, P).transpose(0, 3, 2, 1).astype(np_bf16)
    ).reshape(HK, P, D)
    wgp = np.ascontiguousarray(
        wgj.reshape(D, HK, P).transpose(2, 1, 0).astype(np_bf16)
    ).reshape(P, HK * D)
    wvp = np.ascontiguousarray(
        wvj.reshape(DK, P, HK, P).transpose(0, 3, 2, 1).astype(np_bf16)
    ).reshape(DK, P, H)
    b1p = np.ascontiguousarray(b1j.reshape(HK, P).T.astype(np.float32))
    bgp = np.ascontiguousarray(bgj.reshape(DK, P).T.astype(np.float32))
    bvp = np.ascontiguousarray(bvj.reshape(DK, P).T.astype(np.float32))
    return dict(xT=xT, w1p=w1p, wgp=wgp, wvp=wvp, b1p=b1p, bgp=bgp, bvp=bvp)


def kernel(
    x,
    task_id_tensor,
    task_emb,
    base_gate_w,
    lora_A,
    lora_B,
    w1,
    b1,
    wg,
    bg,
    wv,
    bv,
):
    global last_results, last_exec_time_ns
    x = np.asarray(x)
    bsz, seqlen, dim = x.shape
    assert dim == D

    x_flat, ids, cws = _route(x, task_id_tensor, task_emb, base_gate_w, lora_A, lora_B)

    max_cnt = max(len(i) for i in ids)
    C = max(C_MIN, ((max_cnt + P - 1) // P) * P)
    nc = _get_program(C)

    w1 = np.asarray(w1, dtype=np.float32)
    b1 = np.asarray(b1, dtype=np.float32)
    wg = np.asarray(wg, dtype=np.float32)
    bg = np.asarray(bg, dtype=np.float32)
    wv = np.asarray(wv, dtype=np.float32)
    bv = np.asarray(bv, dtype=np.float32)

    in_maps = [
        _pack_core_inputs(x_flat, ids[j], w1[j], wg[j], wv[j], b1[j], bg[j], bv[j], C)
        for j in range(E)
    ]

    trace = os.environ.get("MOE_TRACE", "0") == "1"
    try:
        res = run_bass_kernel_spmd(nc, in_maps, list(range(E)), trace=trace)
    except (ImportError, ModuleNotFoundError):
        # axon NTFF profiling hook unavailable in this container
        res = run_bass_kernel_spmd(nc, in_maps, list(range(E)), trace=False)
    last_results = res
    last_exec_time_ns = getattr(res, "exec_time_ns", None)

    out_flat = np.zeros((bsz * seqlen, D), dtype=np.float32)
    for j in range(E):
        cnt = len(ids[j])
        if cnt == 0:
            continue
        e = np.asarray(res.results[j]["outT"]).reshape(D, C)[:, :cnt].T
        out_flat[ids[j]] += cws[j][:, None] * e
    return out_flat.reshape(bsz, seqlen, dim)
